# revision 36
# baseline (speedup 1.0000x reference)
"""Trainium2 Bass kernel for nn_ATNLPmodel (retrieval_knn).

Math: the reference builds one-hot "snapshots" snaps[b,r,c,l] = (seq[b, idx[b,r,l]] == c)
with idx[b,r,l] = floor(kp_start[b,r] + kp_len[b,r] * l/(L2-1)), then computes
    act[b,k] = sum_r sum_{c,l} snaps[b,r,c,l] * db[k,c,l].
The sum over r folds into S[b, cl] = sum_r snaps[b,r,cl]  (a [4, 512] count matrix),
so      act = S @ db_flat.T          with db_flat = db.reshape(K, 512).

Strategy: shard the database K=200000 across 8 cores (25000 rows each, padded to
25088 = 49*512). Each core computes S on-device from the raw seq/kp inputs
(tiny: broadcast, fma, floor, indirect-gather, one-hot compare, reduce), then
streams its db shard from HBM through the PE as the moving matmul operand
against the stationary S columns. The shard is laid out host-side transposed
(class-major -> [contraction, k] tiles) so every DMA is fully contiguous
2KB-per-partition lines and no on-chip transposes are needed. Memory-bound:
~51MB HBM traffic per core at ~358 GB/s -> ~143us roofline.
"""

import sys
import numpy as np

for _p in ("/opt/trn_rl_repo",):
    if _p not in sys.path:
        sys.path.insert(0, _p)

import concourse.bass as bass
import concourse.bacc as bacc
import concourse.mybir as mybir
import concourse.tile as tile

F32 = mybir.dt.float32
F32R = mybir.dt.float32r
BF16 = mybir.dt.bfloat16
F16 = mybir.dt.float16
I32 = mybir.dt.int32

B, L1, R, K = 4, 2048, 4, 200000
C, L2 = 32, 16
CL = C * L2                      # 512 contraction
N_CORES = 8
K_SHARD = K // N_CORES           # 25000
N_MACRO = 49                     # k tiles of 512 per core
K_PAD = N_MACRO * 512            # 25088
MM_DTYPE = F32R                  # moving/stationary dtype for the big matmul


def build_kernel(mm_dtype=MM_DTYPE, group=7, raw_bufs=3, out_bufs=2, reps=1,
                 skip_dma=False, skip_mm=False, skip_store=False,
                 batch_store=True, psum_bufs=4, contig=True, split=False,
                 store_engine="scalar", dma_parts=1, alt_load=False):
    assert N_MACRO % group == 0
    n_dma = N_MACRO // group
    nplanes = 2 if split else 1
    if split:
        mm_dtype = BF16
    nc = bacc.Bacc(None, target_bir_lowering=False)

    seq_d = nc.dram_tensor("seq", [B * L1, 1], I32, kind="ExternalInput")
    kpv_d = nc.dram_tensor("kpv", [1, 48], F32, kind="ExternalInput")
    if contig:
        dbt_d = nc.dram_tensor(
            "dbt", [n_dma, 128, group, 4, nplanes, 512], mm_dtype,
            kind="ExternalInput",
        )
    else:
        assert not split
        dbt_d = nc.dram_tensor(
            "dbt", [N_MACRO, 4, 128, 1, 512], mm_dtype, kind="ExternalInput"
        )
    out_d = nc.dram_tensor("out", [B, K_PAD], F32, kind="ExternalOutput")

    def load_ap(d):
        if contig:
            return dbt_d[d]
        return dbt_d[d * group : (d + 1) * group].rearrange("g c p o k -> p g c o k")

    # constants baked into the NEFF
    frac_np = (np.arange(L2, dtype=np.float32) / np.float32(L2 - 1)).reshape(L2, 1)
    frac_c = nc.inline_tensor(frac_np, name="frac_c")
    p_idx = np.arange(128)
    cvals_np = np.stack([(ci * 128 + p_idx) // L2 for ci in range(4)], axis=1).astype(np.float32)
    cvals_c = nc.inline_tensor(cvals_np, name="cvals_c")

    with tile.TileContext(nc) as tc:
        with (
            tc.tile_pool(name="spool", bufs=1) as spool,
            tc.tile_pool(name="eqpool", bufs=2) as eqpool,
            tc.tile_pool(name="raw", bufs=raw_bufs) as rawpool,
            tc.tile_pool(name="outp", bufs=out_bufs) as outpool,
            tc.tile_pool(name="psp", bufs=psum_bufs, space="PSUM") as psp,
        ):
            # ---- prologue: S_T [128 x 16] (4 chunks of [cl-part, b]) ----
            kp_all = spool.tile([16, 48], F32)
            nc.sync.dma_start(kp_all[:], kpv_d[0:1, :].to_broadcast([16, 48]))
            frac_sb = spool.tile([L2, 1], F32)
            nc.sync.dma_start(frac_sb[:], frac_c[:])

            # pos[l, j=(b,r)] = len[j]*frac[l] + start[j] + b*2048  (all f32 exact-ordered)
            t1 = spool.tile([16, 16], F32)
            nc.vector.tensor_scalar_mul(t1[:], kp_all[:, 16:32], frac_sb[:, 0:1])
            t2 = spool.tile([16, 16], F32)
            nc.vector.tensor_tensor(t2[:], t1[:], kp_all[:, 0:16], op=mybir.AluOpType.add)
            pos = spool.tile([16, 16], F32)
            nc.vector.tensor_tensor(pos[:], t2[:], kp_all[:, 32:48], op=mybir.AluOpType.add)

            # goff = floor(pos) robust to any f32->i32 rounding mode
            gi = spool.tile([16, 16], I32)
            nc.vector.tensor_copy(gi[:], pos[:])
            gf = spool.tile([16, 16], F32)
            nc.vector.tensor_copy(gf[:], gi[:])
            over = spool.tile([16, 16], I32)
            nc.vector.tensor_tensor(over[:], gf[:], pos[:], op=mybir.AluOpType.is_gt)
            goff = spool.tile([16, 16], I32)
            nc.vector.tensor_tensor(goff[:], gi[:], over[:], op=mybir.AluOpType.subtract)

            # gather tokens: tokT[l, j] = seq_flat[goff[l, j]]. HW indirect DMA
            # honors one offset per partition (row gather), so gather column-wise.
            tokT = spool.tile([16, 16], I32)
            for j in range(16):
                nc.gpsimd.indirect_dma_start(
                    out=tokT[:, j : j + 1],
                    out_offset=None,
                    in_=seq_d[:],
                    in_offset=bass.IndirectOffsetOnAxis(ap=goff[:, j : j + 1], axis=0),
                )
            tok_all = spool.tile([128, 16], I32)
            for g in range(8):
                nc.sync.dma_start(tok_all[g * 16 : (g + 1) * 16, :], tokT[:])

            cv = spool.tile([128, 4], F32)
            nc.sync.dma_start(cv[:], cvals_c[:])
            tokf = spool.tile([128, 16], F32)
            nc.vector.tensor_copy(tokf[:], tok_all[:])

            # one-hot compare + reduce over r -> S counts
            s_f = spool.tile([128, 16], F32)
            for ci in range(4):
                eq_t = eqpool.tile([128, 16], F32)
                nc.vector.tensor_scalar(
                    eq_t[:], tokf[:], cv[:, ci : ci + 1], None,
                    op0=mybir.AluOpType.is_equal,
                )
                nc.vector.tensor_reduce(
                    s_f[:, ci * 4 : (ci + 1) * 4],
                    eq_t[:].rearrange("p (b r) -> p b r", r=R),
                    axis=mybir.AxisListType.X,
                    op=mybir.AluOpType.add,
                )
            if mm_dtype == F32:
                s_r = s_f
            else:
                s_r = spool.tile([128, 16], mm_dtype)
                nc.vector.tensor_copy(s_r[:], s_f[:])
                # S counts are small ints: exact in every supported dtype.

            # ---- main loop: stream db shard, accumulate act into PSUM ----
            stat = None
            tile_shape = [128, group, 4, nplanes, 512]
            if skip_dma:
                stat = spool.tile(tile_shape, mm_dtype, tag="stat")
                nc.sync.dma_start(stat[:], load_ap(0))

            def main_body():
                for d in range(n_dma):
                    if skip_dma:
                        raw = stat
                    else:
                        raw = rawpool.tile(tile_shape, mm_dtype, tag="raw")
                        if dma_parts == 1:
                            nc.sync.dma_start(raw[:], load_ap(d))
                        else:
                            ap = load_ap(d)
                            bnds = [
                                group * i // dma_parts for i in range(dma_parts + 1)
                            ]
                            for i in range(dma_parts):
                                eng = (
                                    nc.scalar if (alt_load and i % 2) else nc.sync
                                )
                                eng.dma_start(
                                    raw[:, bnds[i] : bnds[i + 1]],
                                    ap[:, bnds[i] : bnds[i + 1]],
                                )
                    outg = None
                    if batch_store and not (skip_mm or skip_store):
                        outg = outpool.tile([B, group * 512], F32, tag="outg")
                    for gi in range(group):
                        m = d * group + gi
                        if skip_mm:
                            continue
                        ps = psp.tile([B, 512], F32, tag="ps")
                        n_mm = 4 * nplanes
                        for mmix, (ci, pl) in enumerate(
                            (c, p) for c in range(4) for p in range(nplanes)
                        ):
                            nc.tensor.matmul(
                                ps[:],
                                lhsT=s_r[:, ci * 4 : (ci + 1) * 4],
                                rhs=raw[:, gi, ci, pl, :],
                                start=(mmix == 0),
                                stop=(mmix == n_mm - 1),
                            )
                        if skip_store:
                            continue
                        st_eng = getattr(nc, store_engine)
                        if batch_store:
                            nc.vector.tensor_copy(
                                outg[:, gi * 512 : (gi + 1) * 512], ps[:]
                            )
                        else:
                            out_t = outpool.tile([B, 512], F32, tag="out_t")
                            nc.vector.tensor_copy(out_t[:], ps[:])
                            st_eng.dma_start(
                                out_d[:, m * 512 : (m + 1) * 512], out_t[:]
                            )
                    if outg is not None:
                        getattr(nc, store_engine).dma_start(
                            out_d[:, d * group * 512 : (d + 1) * group * 512], outg[:]
                        )

            if reps == 1:
                main_body()
            else:
                with tc.For_i(0, reps, 1):
                    main_body()

    nc.compile()
    return nc


def prep_inputs(seq_input, kp_start, kp_len, database, group=7, contig=True,
                np_dtype=np.float32, split=False):
    """Host-side marshaling: dtype casts, packing, shard layout."""
    n_dma = N_MACRO // group
    seq = np.ascontiguousarray(
        np.asarray(seq_input).astype(np.int32).reshape(B * L1, 1)
    )
    st = np.asarray(kp_start).astype(np.float32).reshape(-1)
    ln = np.asarray(kp_len).astype(np.float32).reshape(-1)
    bo = ((np.arange(16) // R) * L1).astype(np.float32)
    kpv = np.concatenate([st, ln, bo]).reshape(1, 48).astype(np.float32)

    db = np.asarray(database, dtype=np.float32).reshape(K, CL)
    in_maps = []
    for i in range(N_CORES):
        shard = db[i * K_SHARD : (i + 1) * K_SHARD]
        pad = np.zeros((K_PAD, CL), dtype=np.float32)
        pad[:K_SHARD] = shard
        if split:
            import ml_dtypes
            hi = pad.astype(ml_dtypes.bfloat16)
            lo = (pad - hi.astype(np.float32)).astype(ml_dtypes.bfloat16)
            # planes [K_PAD, CL, 2]
            planes = np.stack([hi, lo], axis=-1)
            # dbt[d, p, g, c, o, jj] = planes[(d*group+g)*512+jj, c*128+p, o]
            dbt = np.ascontiguousarray(
                planes.reshape(n_dma, group, 512, 4, 128, 2).transpose(0, 4, 1, 3, 5, 2)
            )
        elif contig:
            # dbt[d, p, g, c, 0, jj] = pad[(d*group+g)*512 + jj, c*128 + p]
            dbt = np.ascontiguousarray(
                pad.astype(np_dtype)
                .reshape(n_dma, group, 512, 4, 128)
                .transpose(0, 4, 1, 3, 2)[:, :, :, :, None, :]
            )
        else:
            # dbt[m, c, p, 0, jj] = pad[m*512+jj, c*128+p]
            dbt = np.ascontiguousarray(
                pad.astype(np_dtype)
                .reshape(N_MACRO, 512, 4, 128)
                .transpose(0, 2, 3, 1)[:, :, :, None, :]
            )
        in_maps.append({"seq": seq, "kpv": kpv, "dbt": dbt})
    return in_maps


_NC_CACHE = {}

# Ship configuration: bf16 hi/lo split streams the same 4 bytes/element as
# f32 (DMA-bound either way) but runs the PE at bf16 rate with ~3e-6 overall
# relative error (vs ~1e-4 for f32r, ~2.3e-7 for true fp32 at +43% time).
# Measured steady state ~155us/core vs ~147us pure-DMA floor (349 GB/s).
SHIP_BUILD = dict(split=True, dma_parts=7, raw_bufs=2)
SHIP_PREP = dict(group=7, contig=True, split=True)


def kernel(seq_input, kp_start, kp_len, database):
    import time
    from concourse.bass_utils import run_bass_kernel_spmd

    if "nc" not in _NC_CACHE:
        _NC_CACHE["nc"] = build_kernel(**SHIP_BUILD)
    nc = _NC_CACHE["nc"]
    in_maps = prep_inputs(seq_input, kp_start, kp_len, database, **SHIP_PREP)
    res = None
    for attempt in range(3):
        try:
            res = run_bass_kernel_spmd(nc, in_maps, core_ids=list(range(N_CORES)))
            break
        except Exception:
            if attempt == 2:
                raise
            time.sleep(5)
    out = np.concatenate(
        [res.results[i]["out"][:, :K_SHARD] for i in range(N_CORES)], axis=1
    )
    return np.ascontiguousarray(out.astype(np.float32))


if __name__ == "__main__":
    # CoreSim self-check against a host recomputation on synthetic data.
    from concourse.bass_interp import CoreSim

    rng = np.random.default_rng(1)
    seq_input = rng.integers(0, C, (B, L1)).astype(np.int64)
    kp_start = np.sort(rng.integers(0, L1 - 257, (B, R)), axis=-1).astype(np.int64)
    kp_len = (rng.integers(0, 255, (B, R)) + 1).astype(np.int64)
    database = rng.standard_normal((K, C, L2)).astype(np.float32)

    # host reference (mirrors reference.py in fp32)
    frac = np.arange(L2, dtype=np.float32) / np.float32(L2 - 1)
    pos = kp_start.astype(np.float32)[..., None] + kp_len.astype(np.float32)[..., None] * frac
    idx = np.clip(np.floor(pos).astype(np.int64), 0, L1 - 1)
    snaps = np.zeros((B, R, C, L2), dtype=np.float32)
    for b in range(B):
        for r in range(R):
            for l in range(L2):
                snaps[b, r, seq_input[b, idx[b, r, l]], l] = snaps[b, r, seq_input[b, idx[b, r, l]], l] + 1
    S = snaps.sum(axis=1).reshape(B, CL)
    ref = S @ database.reshape(K, CL).T

    nc = build_kernel(**SHIP_BUILD)
    in_maps = prep_inputs(seq_input, kp_start, kp_len, database, **SHIP_PREP)
    core = int(sys.argv[1]) if len(sys.argv) > 1 else 0
    sim = CoreSim(nc)
    for name, val in in_maps[core].items():
        sim.tensor(name)[:] = val
    sim.simulate()
    got = np.array(sim.tensor("out"))[:, :K_SHARD]
    want = ref[:, core * K_SHARD : (core + 1) * K_SHARD]
    err = np.abs(got - want).max() / max(np.abs(want).max(), 1e-9)
    print(f"CoreSim core {core}: rel err {err:.3e}")
    assert err < 1e-5, "sim mismatch"
    print("SIM OK")
